# revision 1
# baseline (speedup 1.0000x reference)
"""Trainium2 Bass kernel for nn_CondFilterT (embedding lookup + cosine filter).

Computation (per batch row b):
    e   = table[input[b, 0]]                      # [64] raw event embedding
    c_j = table[input[b, 1+j]]  j in [0, 50)      # [64] condition embeddings
    en  = e / ||e||;  cn_j = c_j / ||c_j||
    s_j = en . cn_j
    out[b] = concat(e, cn_0 * s_0, ..., cn_49 * s_49)   # [51*64] = [3264]

Sharding: data-parallel over batch across 8 NeuronCores; the 25.6MB f32
embedding table is replicated per core in HBM and rows are fetched with
indirect (gather) DMA.

Identity used on-chip:  cn_j * s_j = c_j * (inv_j^2 * inv_0 * (e . c_j))
with inv = 1/||.||, so the gathered raw tile can be scaled in place and
written out directly.
"""

import numpy as np

NCORES = 8
B = 16384
C = 51  # 1 event + 50 conditions
CC = C - 1  # 50 conditions
D = 64
V = 100002
ROWS_PER_CORE = B // NCORES  # 2048
P = 128
NTILES = ROWS_PER_CORE // P  # 16

_CACHE = {}


def _build_nc(rows=ROWS_PER_CORE, v=V, embufs=3, tmpbufs=2, smallbufs=3):
    import concourse.bass as bass
    import concourse.bacc as bacc
    import concourse.tile as tile
    from concourse import mybir

    ntiles = rows // P
    nc = bacc.Bacc(None)

    idx_ext = nc.declare_dram_parameter(
        "idx", [rows, C], mybir.dt.int32, isOutput=False
    )
    tab_ext = nc.declare_dram_parameter(
        "table", [v, D], mybir.dt.float32, isOutput=False
    )
    out_ext = nc.declare_dram_parameter(
        "out", [rows, C * D], mybir.dt.float32, isOutput=True
    )

    FP = mybir.dt.float32
    MUL = mybir.AluOpType.mult
    ADD = mybir.AluOpType.add
    AX = mybir.AxisListType.X

    with tile.TileContext(nc) as tc:
        with (
            tc.tile_pool(name="emb", bufs=embufs) as emb_pool,
            tc.tile_pool(name="tmp", bufs=tmpbufs) as tmp_pool,
            tc.tile_pool(name="small", bufs=smallbufs) as small_pool,
            tc.tile_pool(name="idx", bufs=ntiles) as idx_pool,
        ):
            for t in range(ntiles):
                r0 = t * P

                idx_t = idx_pool.tile([P, C], mybir.dt.int32)
                nc.sync.dma_start(out=idx_t[:], in_=idx_ext[r0 : r0 + P, :])

                # Gather 128*51 table rows; row (p, j) -> E[p, j, :].
                # HW indirect DMA supports ONE dynamic offset per partition,
                # so issue one gather per lookup column j.
                E = emb_pool.tile([P, C, D], FP)
                for j in range(C):
                    nc.gpsimd.indirect_dma_start(
                        out=E[:, j, :],
                        out_offset=None,
                        in_=tab_ext[:, :],
                        in_offset=bass.IndirectOffsetOnAxis(
                            ap=idx_t[:, j : j + 1], axis=0
                        ),
                    )

                # Sum of squares per embedding -> norms.
                T1 = tmp_pool.tile([P, C, D], FP, tag="sq")
                nc.scalar.square(out=T1[:], in_=E[:])
                ss = small_pool.tile([P, C], FP)
                nc.vector.tensor_reduce(out=ss[:], in_=T1[:], axis=AX, op=ADD)
                nrm = small_pool.tile([P, C], FP)
                nc.scalar.sqrt(out=nrm[:], in_=ss[:])
                inv = small_pool.tile([P, C], FP)
                nc.vector.reciprocal(out=inv[:], in_=nrm[:])

                # Raw dot products e . c_j.
                T2 = tmp_pool.tile([P, CC, D], FP, tag="prod")
                nc.vector.tensor_tensor(
                    out=T2[:],
                    in0=E[:, 1:, :],
                    in1=E[:, 0:1, :].to_broadcast([P, CC, D]),
                    op=MUL,
                )
                rd = small_pool.tile([P, CC], FP)
                nc.vector.tensor_reduce(out=rd[:], in_=T2[:], axis=AX, op=ADD)

                # g_j = inv_j^2 * inv_0 * (e . c_j)
                g = small_pool.tile([P, CC], FP)
                nc.vector.tensor_mul(out=g[:], in0=inv[:, 1:], in1=inv[:, 1:])
                nc.vector.tensor_mul(out=g[:], in0=g[:], in1=rd[:])
                nc.vector.tensor_scalar_mul(out=g[:], in0=g[:], scalar1=inv[:, 0:1])

                # Scale condition rows in place; event row stays raw.
                nc.vector.tensor_tensor(
                    out=E[:, 1:, :],
                    in0=E[:, 1:, :],
                    in1=g[:].unsqueeze(2).to_broadcast([P, CC, D]),
                    op=MUL,
                )

                nc.sync.dma_start(
                    out=out_ext[r0 : r0 + P, :],
                    in_=E[:].rearrange("p c d -> p (c d)"),
                )

    nc.finalize()
    return nc


def _get_nc():
    if "nc" not in _CACHE:
        _CACHE["nc"] = _build_nc()
    return _CACHE["nc"]


def _run(input, event_table, trace=False, **spmd_kwargs):
    from concourse.bass_utils import run_bass_kernel_spmd

    idx = np.ascontiguousarray(np.asarray(input).astype(np.int32))
    tab = np.ascontiguousarray(np.asarray(event_table, dtype=np.float32))
    assert idx.shape == (B, C), idx.shape
    assert tab.shape == (V, D), tab.shape

    nc = _get_nc()
    in_maps = [
        {
            "idx": idx[c * ROWS_PER_CORE : (c + 1) * ROWS_PER_CORE],
            "table": tab,
        }
        for c in range(NCORES)
    ]
    res = run_bass_kernel_spmd(
        nc, in_maps, core_ids=list(range(NCORES)), trace=trace, **spmd_kwargs
    )
    out = np.concatenate([r["out"] for r in res.results], axis=0)
    return out, res


def kernel(input, event_table):
    out, _ = _run(input, event_table)
    return out

